# revision 1
# baseline (speedup 1.0000x reference)
"""Trainium2 Bass kernel for nn_CollectiveDecActorTaxi0Obs (gnn_message_passing).

Computes, for obs [32768, 48], per-zone dense heads W [81, 48, 5] (+bias b,
adjacency idx/mask [81, 5]):
    logits = einsum('bd,ndk->bnk', obs, W) + b ; masked softmax over k
    out[b, n, idx[n, k]] += probs[b, n, k]              -> [32768, 81, 81] f32

Strategy (pure data parallelism, 8 cores, batch-sharded 4096 rows each):
  All small operands (W, b, idx, mask) are folded on the host into constant
  matrices so the device only runs matmuls + exp + elementwise:
    - Wa [49, 448]:   W flattened to padded slot columns with a bias row
                      appended; masked slots get bias -1e9 (exp underflows to
                      exactly 0, matching the reference's where(mask>0,.,-1e9)).
    - ob_p [pw, 81]:  0/1 slot->zone map -> per-zone sums of exp (softmax den)
    - E [81, 448]:    expands per-zone reciprocal denom back to slot rows
    - S [128, 6561]:  0/1 selection matrix built from idx; the scatter into
                      the 81-wide adjacency vector IS a matmul probs @ S
                      (duplicate idx entries accumulate, like .at[].add).
  fp32 matmuls on TRN2 cost 2 weight passes x 2 cycles/col; bf16 costs 1 x 1.
  probs is split hi+lo into two bf16 tensors (x == hi + lo to ~2^-18 relative)
  that are STACKED on the contraction axis: since both multiply the same 0/1
  S matrix (exact in bf16), one K=128 bf16 matmul computes hi@S + lo@S at a
  quarter of the fp32 cost (matmul time scales with N only). The same split
  handles the recip-denominator expansion. The softmax denominator matmul
  stays fp32 for accuracy; its reciprocal runs on the vector engine.

  Slot layout: 14 scatter groups of 6 zones (30 slots; last group 3 zones),
  two groups -> one 64-row half-chunk [A|pad|B], two half-chunks -> one
  128-row pair for the fp32 logits/den stage. The split tiles pcat hold the
  half-chunk's hi rows at 0..63 and lo rows at 64..127, so every scatter
  matmul is a full-K (128) single pass whose unused rows hit zero S rows.

  Everything runs in a transposed layout (batch on the free dim) until the
  scatter matmul, whose PSUM output lands batch-on-partitions so dense
  [128, 6561] tiles stream to DRAM with unit-stride rows.
  The kernel is HBM-write-bound: 860 MB of output, ~107 MB/core, ~320 us
  at the ~358 GB/s per-core HBM limit.
"""

import os
import sys

sys.path.insert(0, "/opt/trn_rl_repo")

import numpy as np

NZ = 81          # zones
D = 48           # obs dim used
DA = D + 1       # + bias row
KADJ = 5         # adjacency slots per zone
NCORES = 8
BATCH = 32768
BLOC = BATCH // NCORES   # 4096 rows per core
BF = 512                 # batch free-dim block (matmul N limit for fp32 PSUM)
P = 128
NEG = np.float32(-1e9)

ZPG = 6                        # zones per scatter group (30 slots + 2 pad)
NGRP = 14                      # groups: 13x6 zones + 1x3 zones
GRP_NZ = [6] * 13 + [3]
GRP_COL = [486 * g for g in range(14)]          # output column offset
PW_PAIR = [128, 128, 128, 64]  # used rows per pair (pair 3 = one half-chunk)
PADW = 448                     # 3*128 + 64 packed columns

LAST_RESULTS = None


def _slot(n, k):
    """(zone, k) -> (pair, row_in_pair, halfchunk, row_in_halfchunk_hi)."""
    g = n // ZPG
    zz = n % ZPG
    hc = g // 2
    p = hc // 2
    row_hi = 32 * (g % 2) + KADJ * zz + k       # 0..61 within half-chunk
    row_pair = 64 * (hc % 2) + row_hi
    return p, row_pair, hc, row_hi


def _build_consts(W, b, idx, mask):
    import ml_dtypes

    bf = ml_dtypes.bfloat16
    W = np.asarray(W, np.float32)
    b = np.asarray(b, np.float32)
    idx = np.asarray(idx)
    mask = np.asarray(mask, np.float32)

    Wa = np.zeros((DA, PADW), np.float32)
    E = np.zeros((NZ, PADW), bf)
    ob = [np.zeros((PW_PAIR[p], NZ), np.float32) for p in range(4)]
    S = np.zeros((P, NZ * NZ), bf)

    for n in range(NZ):
        for k in range(KADJ):
            p, rp, hc, rh = _slot(n, k)
            col = 128 * p + rp
            if mask[n, k] > 0:
                Wa[:D, col] = W[n, :, k]
                Wa[D, col] = b[n, k]
            else:
                Wa[D, col] = NEG
            E[n, col] = 1.0
            ob[p][rp, n] = 1.0
            ocol = n * NZ + int(idx[n, k])
            S[rh, ocol] = 1.0        # hi rows
            S[64 + rh, ocol] = 1.0   # lo rows
    return Wa, E, ob, S


def _build_program(bloc):
    from concourse import bacc, mybir
    import concourse.tile as tile

    f32 = mybir.dt.float32
    bf16 = mybir.dt.bfloat16
    AF = mybir.ActivationFunctionType
    OP = mybir.AluOpType
    nc = bacc.Bacc("TRN2", target_bir_lowering=False, debug=False)

    xTa_d = nc.declare_dram_parameter("xTa", [DA, bloc], f32, isOutput=False)
    Wa_d = nc.declare_dram_parameter("Wa", [DA, PADW], f32, isOutput=False)
    E_d = nc.declare_dram_parameter("E", [NZ, PADW], bf16, isOutput=False)
    ob_d = [
        nc.declare_dram_parameter(f"ob{p}", [PW_PAIR[p], NZ], f32, isOutput=False)
        for p in range(4)
    ]
    S_d = nc.declare_dram_parameter("S", [P, NZ * NZ], bf16, isOutput=False)
    out_d = nc.declare_dram_parameter("out", [bloc, NZ * NZ], f32, isOutput=True)

    n_blk = bloc // BF
    n_sub = BF // P

    with tile.TileContext(nc) as tc:
        with (
            tc.tile_pool(name="const", bufs=1) as cpool,
            tc.tile_pool(name="work", bufs=2) as wpool,
            tc.tile_pool(name="outp", bufs=4) as opool,
            tc.tile_pool(name="ps_log", bufs=2, space="PSUM") as ps_log,
            tc.tile_pool(name="ps_den", bufs=1, space="PSUM") as ps_den,
            tc.tile_pool(name="ps_rf", bufs=2, space="PSUM") as ps_rf,
            tc.tile_pool(name="ps_sc", bufs=3, space="PSUM") as ps_sc,
        ):
            Wa_sb = cpool.tile([DA, PADW], f32, tag="Wa")
            nc.sync.dma_start(out=Wa_sb[:], in_=Wa_d[:])
            E_sb = cpool.tile([NZ, PADW], bf16, tag="E")
            nc.sync.dma_start(out=E_sb[:], in_=E_d[:])
            S_sb = cpool.tile([P, NZ * NZ], bf16, tag="S")
            nc.sync.dma_start(out=S_sb[:], in_=S_d[:])
            ob_sb = []
            for p in range(4):
                t = cpool.tile([PW_PAIR[p], NZ], f32, tag=f"ob{p}")
                nc.sync.dma_start(out=t[:], in_=ob_d[p][:])
                ob_sb.append(t)
            xTa_sb = cpool.tile([DA, bloc], f32, tag="xTa")
            nc.sync.dma_start(out=xTa_sb[:], in_=xTa_d[:])

            def emit_scatter(bs, pcat):
                for i in range(n_sub):
                    osb = opool.tile([P, NZ * NZ], f32, tag="osb")
                    for g in range(NGRP):
                        ncols = GRP_NZ[g] * NZ
                        colg = GRP_COL[g]
                        sc = ps_sc.tile([P, BF], f32, tag="scps")
                        nc.tensor.matmul(
                            sc[:, :ncols],
                            pcat[g // 2][:, i * P:(i + 1) * P],
                            S_sb[:, colg:colg + ncols],
                            start=True,
                            stop=True,
                        )
                        dst = osb[:, colg:colg + ncols]
                        if g % 5 < 3:
                            nc.scalar.copy(dst, sc[:, :ncols])
                        else:
                            nc.vector.tensor_copy(dst, sc[:, :ncols])
                    nc.sync.dma_start(
                        out=out_d[bs + i * P: bs + (i + 1) * P, :], in_=osb[:]
                    )

            prev = None
            for blk in range(n_blk):
                bs = blk * BF
                exT = []
                for p in range(4):
                    pw = PW_PAIR[p]
                    lg = ps_log.tile([P, BF], f32, tag="lg")
                    nc.tensor.matmul(
                        lg[:pw, :],
                        Wa_sb[:, 128 * p:128 * p + pw],
                        xTa_sb[:, bs:bs + BF],
                        start=True,
                        stop=True,
                    )
                    ex = wpool.tile([P, BF], f32, tag=f"exp{p}")
                    nc.scalar.activation(ex[:pw, :], lg[:pw, :], AF.Exp)
                    exT.append(ex)
                den_ps = ps_den.tile([NZ, BF], f32, tag="den")
                for p in range(4):
                    nc.tensor.matmul(
                        den_ps[:, :], ob_sb[p][:], exT[p][:PW_PAIR[p], :],
                        start=(p == 0), stop=(p == 3),
                    )
                rc = wpool.tile([NZ, BF], f32, tag="recipC")
                nc.vector.reciprocal(rc[:], den_ps[:])
                rhi = wpool.tile([NZ, BF], bf16, tag="rhi")
                nc.scalar.copy(rhi[:], rc[:])
                rlo = wpool.tile([NZ, BF], bf16, tag="rlo")
                nc.vector.tensor_tensor(out=rlo[:], in0=rc[:], in1=rhi[:], op=OP.subtract)
                pcat = []
                for p in range(4):
                    pw = PW_PAIR[p]
                    rf = ps_rf.tile([P, BF], f32, tag="rf")
                    nc.tensor.matmul(
                        rf[:pw, :], E_sb[:, 128 * p:128 * p + pw], rhi[:],
                        start=True, stop=False,
                    )
                    nc.tensor.matmul(
                        rf[:pw, :], E_sb[:, 128 * p:128 * p + pw], rlo[:],
                        start=False, stop=True,
                    )
                    for h in range(2 if pw == 128 else 1):
                        sl = slice(64 * h, 64 * h + 64)
                        pt = wpool.tile([64, BF], f32, tag=f"pt{2 * p + h}")
                        nc.vector.tensor_tensor(
                            out=pt[:, :], in0=exT[p][sl, :], in1=rf[sl, :], op=OP.mult
                        )
                        pc = wpool.tile([P, BF], bf16, tag=f"pcat{2 * p + h}")
                        nc.scalar.copy(pc[:64, :], pt[:, :])
                        nc.vector.tensor_tensor(
                            out=pc[64:, :],
                            in0=pt[:, :],
                            in1=pc[:64, :],
                            op=OP.subtract,
                        )
                        pcat.append(pc)
                if prev is not None:
                    emit_scatter(*prev)
                prev = (bs, pcat)
            emit_scatter(*prev)
    nc.compile()
    return nc


def _install_ntff_hook():
    """Shim antenv.axon_hooks (absent in this image) so trace=True can drive
    NRT profiling through libaxon_pjrt.so. Only used for self-profiling."""
    import types

    try:
        import antenv

        try:
            from antenv.axon_hooks import get_axon_ntff_profile_hook  # noqa: F401

            return True
        except ImportError:
            pass
        if "/root/.axon_site" not in sys.path:
            sys.path.insert(0, "/root/.axon_site")
        from trn_agent_boot.trn_boot import _ntff_profile_via_ctypes

        hook = _ntff_profile_via_ctypes("/opt/axon/libaxon_pjrt.so")
        mod = types.ModuleType("antenv.axon_hooks")
        state = {"hook": hook}
        mod.get_axon_ntff_profile_hook = lambda: state["hook"]
        mod.set_axon_ntff_profile_hook = lambda h: state.update(hook=h)
        sys.modules["antenv.axon_hooks"] = mod
        antenv.axon_hooks = mod
        return hook is not None
    except Exception as e:  # profiling is best-effort; never break the run
        print("ntff hook install failed:", e)
        return False


def kernel(obs, W, b, idx, mask):
    from concourse.bass_utils import run_bass_kernel_spmd

    global LAST_RESULTS
    trace = bool(int(os.environ.get("KBT_TRACE", "0")))
    if trace:
        trace = _install_ntff_hook()
    obs = np.asarray(obs, np.float32)
    Wa, E, ob, S = _build_consts(W, b, idx, mask)

    nc = _build_program(BLOC)

    consts = {"Wa": Wa, "E": E, "S": S}
    for p in range(4):
        consts[f"ob{p}"] = ob[p]

    in_maps = []
    for i in range(NCORES):
        shard = obs[i * BLOC:(i + 1) * BLOC, :D]
        xTa = np.concatenate(
            [np.ascontiguousarray(shard.T), np.ones((1, BLOC), np.float32)], axis=0
        )
        m = dict(consts)
        m["xTa"] = np.ascontiguousarray(xTa)
        in_maps.append(m)

    br = run_bass_kernel_spmd(nc, in_maps, list(range(NCORES)), trace=trace)
    LAST_RESULTS = br
    out = np.concatenate([br.results[i]["out"] for i in range(NCORES)], axis=0)
    return out.reshape(BATCH, NZ, NZ)



# revision 4
# speedup vs baseline: 1.6244x; 1.6244x over previous
"""Trainium2 Bass kernel for nn_CollectiveDecActorTaxi0Obs (gnn_message_passing).

Computes, for obs [32768, 48], per-zone dense heads W [81, 48, 5] (+bias b,
adjacency idx/mask [81, 5]):
    logits = einsum('bd,ndk->bnk', obs, W) + b ; masked softmax over k
    out[b, n, idx[n, k]] += probs[b, n, k]              -> [32768, 81, 81] f32

Strategy (pure data parallelism, 8 cores, batch-sharded 4096 rows each):

FAST PATH (taken when idx has diagonal structure, i.e. the per-zone offsets
delta = idx[n,k] - n fall into a small shared class set with no duplicate
targets -- true for the 9x9 grid adjacency: delta in {-9,-1,0,+1,+9}):
  * Slots are laid out (class, zone): slot s = c*81 + n. Absent/masked slots
    get bias -1e9 so exp underflows to exactly 0 (matching the reference's
    where(mask>0, ., -1e9) + softmax*mask).
  * logits via two bf16 matmuls (hi/lo split of both x and W) accumulating in
    PSUM -- full fp32-grade accuracy at 1/2 the fp32 matmul cost.
  * softmax denominator: 0/1 "zone-of-slot" matmul (fp32); reciprocal on DVE;
    expanded back to slots by a 0/1 bf16 matmul.
  * probs = exp * recip (DVE), rounded to bf16.
  * PE transpose (vs identity) flips each [slots,128-batch] tile to
    [128-batch, slots] in PSUM.
  * The scatter out[.., n*81+idx] = probs is 5 strided copies per 128-batch
    sub-block: destination column 81*n + idx[n,k] = 82*n + delta, i.e. one
    stride-82 "diagonal" per class. Zero columns of the output tile are
    memset ONCE and never touched again (same columns rewritten every
    iteration), killing the full-width PSUM evacuation the f32 scatter-matmul
    approach needs.
  * Output is stored to DRAM in bf16 (rel err ~2^-9, far under the 2e-2
    tolerance) and upcast to f32 on the host: halves the HBM write traffic,
    which is the roofline for this problem (430 MB @ ~358 GB/s/core).
  * Output DMAs are [128, 4*6561] = 6.7 MB contiguous per 512-batch block
    (batch row = blk*512 + 4*p + j; the batch columns of the on-chip
    transposed layout are pre-permuted on the host so this layout falls out).

FALLBACK (any other idx/mask structure, incl. duplicate targets that must
accumulate): the original fp32 scatter-matmul kernel (probs @ 0/1 S matrix).
"""

import os
import sys

sys.path.insert(0, "/opt/trn_rl_repo")

import numpy as np

NZ = 81          # zones
D = 48           # obs dim used
DA = D + 1       # + bias row
KADJ = 5         # adjacency slots per zone
NCORES = 8
BATCH = 32768
BLOC = BATCH // NCORES   # 4096 rows per core
BF = 512                 # batch free-dim block
P = 128
NEG = np.float32(-1e9)
NN = NZ * NZ             # 6561

LAST_RESULTS = None


# --------------------------------------------------------------------------
# FAST PATH: diagonal-class scatter
# --------------------------------------------------------------------------

def _analyze_classes(idx, mask):
    """Return sorted class offsets (delta = idx - zone) if the adjacency has
    exploitable diagonal structure, else None."""
    idx = np.asarray(idx)
    mask = np.asarray(mask, np.float32)
    deltas = set()
    for n in range(NZ):
        tgts = set()
        nvalid = 0
        for k in range(KADJ):
            if mask[n, k] > 0:
                t = int(idx[n, k])
                if t in tgts:
                    return None          # duplicate target -> must accumulate
                tgts.add(t)
                deltas.add(t - n)
                nvalid += 1
        if nvalid == 0:
            return None                  # zone with no valid slot (den == 0)
    ds = sorted(deltas)
    if len(ds) > 6 or len(ds) * NZ > 512:
        return None
    for a in ds:
        for b in ds:
            if abs(a - b) == 82:         # column-ownership collision
                return None
    return ds


def _build_fast_consts(W, b, idx, mask, deltas):
    import ml_dtypes

    bf = ml_dtypes.bfloat16
    W = np.asarray(W, np.float32)
    b = np.asarray(b, np.float32)
    idx = np.asarray(idx)
    mask = np.asarray(mask, np.float32)

    ncls = len(deltas)
    nslot = ncls * NZ
    nchunk = (nslot + P - 1) // P
    crs = [min(P, nslot - c * P) for c in range(nchunk)]

    # Wa [DA, nslot] fp32: per-slot weight column + bias row
    Wa = np.zeros((DA, nslot), np.float32)
    Wa[D, :] = NEG                        # default: absent slot -> -1e9 bias
    d2c = {d: c for c, d in enumerate(deltas)}
    for n in range(NZ):
        for k in range(KADJ):
            if mask[n, k] > 0:
                s = d2c[int(idx[n, k]) - n] * NZ + n
                Wa[:D, s] = W[n, :, k]
                Wa[D, s] = b[n, k]

    Whi = Wa.astype(bf)
    Wlo = (Wa - Whi.astype(np.float32)).astype(bf)
    WaHH = np.concatenate([Whi, Whi], axis=0)          # [2*DA, nslot] bf16

    # ob chunks [cr, NZ] f32 (zone-of-slot, for the fp32 denominator matmul)
    # E  chunks [NZ, cr] bf16 (slot-of-zone, to expand recip back to slots)
    ob = []
    E = []
    for c in range(nchunk):
        cr = crs[c]
        o = np.zeros((cr, NZ), np.float32)
        e = np.zeros((NZ, cr), bf)
        for l in range(cr):
            n = (c * P + l) % NZ
            o[l, n] = 1.0
            e[n, l] = 1.0
        ob.append(o)
        E.append(e)

    ident = np.eye(P, dtype=np.float32).astype(bf)

    # scatter plan: per class, the n-range whose dest column 82n+delta is
    # in [0, NN)
    plan = []
    for c, dlt in enumerate(deltas):
        n0 = 1 if dlt < 0 else 0
        n1 = 80 if dlt > 0 else 81
        plan.append((c, dlt, n0, n1))
    return WaHH, Wlo, ob, E, ident, plan, crs


def _build_fast_program(bloc, crs, plan):
    from concourse import bacc, mybir
    import concourse.tile as tile

    f32 = mybir.dt.float32
    bf16 = mybir.dt.bfloat16
    AF = mybir.ActivationFunctionType
    OP = mybir.AluOpType
    nc = bacc.Bacc("TRN2", target_bir_lowering=False, debug=False)

    nchunk = len(crs)
    nslot = sum(crs)
    DH = 2 * DA                                   # 98 rows: x hi + x lo
    n_blk = bloc // BF

    xcat_d = nc.declare_dram_parameter("xcat", [DH, bloc], bf16, isOutput=False)
    WaHH_d = nc.declare_dram_parameter("WaHH", [DH, nslot], bf16, isOutput=False)
    WaL_d = nc.declare_dram_parameter("WaL", [DA, nslot], bf16, isOutput=False)
    ob_d = [
        nc.declare_dram_parameter(f"ob{c}", [crs[c], NZ], f32, isOutput=False)
        for c in range(nchunk)
    ]
    E_d = [
        nc.declare_dram_parameter(f"E{c}", [NZ, crs[c]], bf16, isOutput=False)
        for c in range(nchunk)
    ]
    id_d = nc.declare_dram_parameter("ident", [P, P], bf16, isOutput=False)
    out_d = nc.declare_dram_parameter("out", [n_blk, P, 4 * NN], bf16, isOutput=True)

    with tile.TileContext(nc) as tc:
        with (
            tc.tile_pool(name="const", bufs=1) as cpool,
            tc.tile_pool(name="work", bufs=2) as wpool,
            tc.tile_pool(name="ps_log", bufs=2, space="PSUM") as ps_log,
            tc.tile_pool(name="ps_den", bufs=2, space="PSUM") as ps_den,
            tc.tile_pool(name="ps_rf", bufs=2, space="PSUM") as ps_rf,
            tc.tile_pool(name="ps_t", bufs=2, space="PSUM") as ps_t,
        ):
            WaHH_sb = cpool.tile([DH, nslot], bf16, tag="WaHH")
            nc.sync.dma_start(out=WaHH_sb[:], in_=WaHH_d[:])
            WaL_sb = cpool.tile([DA, nslot], bf16, tag="WaL")
            nc.sync.dma_start(out=WaL_sb[:], in_=WaL_d[:])
            ob_sb, E_sb = [], []
            for c in range(nchunk):
                t = cpool.tile([crs[c], NZ], f32, tag=f"ob{c}")
                nc.sync.dma_start(out=t[:], in_=ob_d[c][:])
                ob_sb.append(t)
                t = cpool.tile([NZ, crs[c]], bf16, tag=f"E{c}")
                nc.sync.dma_start(out=t[:], in_=E_d[c][:])
                E_sb.append(t)
            id_sb = cpool.tile([P, P], bf16, tag="ident")
            nc.sync.dma_start(out=id_sb[:], in_=id_d[:])
            xcat_sb = cpool.tile([DH, bloc], bf16, tag="xcat")
            nc.sync.dma_start(out=xcat_sb[:], in_=xcat_d[:])

            osbA = cpool.tile([P, 4 * NN], bf16, tag="osbA")
            nc.vector.memset(osbA[:], 0.0)
            osbB = cpool.tile([P, 4 * NN], bf16, tag="osbB")
            nc.gpsimd.memset(osbB[:], 0.0)
            osbs = [osbA, osbB]

            for blk in range(n_blk):
                bs = blk * BF
                osb = osbs[blk % 2]
                exs = []
                for c in range(nchunk):
                    cr = crs[c]
                    sl = slice(c * P, c * P + cr)
                    lg = ps_log.tile([P, BF], f32, tag="lg")
                    nc.tensor.matmul(
                        lg[:cr, :], WaHH_sb[:, sl], xcat_sb[:, bs:bs + BF],
                        start=True, stop=False,
                    )
                    nc.tensor.matmul(
                        lg[:cr, :], WaL_sb[:, sl], xcat_sb[:DA, bs:bs + BF],
                        start=False, stop=True,
                    )
                    ex = wpool.tile([P, BF], f32, tag=f"ex{c}")
                    nc.scalar.activation(ex[:cr, :], lg[:cr, :], AF.Exp)
                    exs.append(ex)
                den = ps_den.tile([NZ, BF], f32, tag="den")
                for c in range(nchunk):
                    nc.tensor.matmul(
                        den[:, :], ob_sb[c][:], exs[c][:crs[c], :],
                        start=(c == 0), stop=(c == nchunk - 1),
                    )
                rc = wpool.tile([NZ, BF], f32, tag="rc")
                nc.vector.reciprocal(rc[:], den[:])
                rhi = wpool.tile([NZ, BF], bf16, tag="rhi")
                nc.scalar.copy(rhi[:], rc[:])
                pcs = []
                for c in range(nchunk):
                    cr = crs[c]
                    rf = ps_rf.tile([P, BF], f32, tag="rf")
                    nc.tensor.matmul(
                        rf[:cr, :], E_sb[c][:], rhi[:], start=True, stop=True,
                    )
                    pc = wpool.tile([P, BF], bf16, tag=f"pc{c}")
                    nc.vector.tensor_tensor(
                        out=pc[:cr, :], in0=exs[c][:cr, :], in1=rf[:cr, :],
                        op=OP.mult,
                    )
                    pcs.append(pc)
                for j in range(4):
                    pT = ps_t.tile([P, BF], bf16, tag="pT")
                    for c in range(nchunk):
                        cr = crs[c]
                        nc.tensor.transpose(
                            pT[:, c * P:c * P + cr],
                            pcs[c][:cr, j * P:(j + 1) * P],
                            id_sb[:cr, :cr],
                        )
                    for ci, (c, dlt, n0, n1) in enumerate(plan):
                        cnt = n1 - n0
                        src = pT[:, c * NZ + n0:c * NZ + n1]
                        d0 = j * NN + 82 * n0 + dlt
                        dst = osb[:, d0:d0 + 82 * (cnt - 1) + 1:82]
                        if (j + ci) % 2 == 0:
                            nc.vector.tensor_copy(dst, src)
                        else:
                            nc.scalar.copy(dst, src)
                nc.sync.dma_start(out=out_d[blk], in_=osb[:])
    nc.compile()
    return nc


def _run_fast(obs, W, b, idx, mask, deltas, trace):
    import ml_dtypes
    from concourse.bass_utils import run_bass_kernel_spmd

    global LAST_RESULTS
    bf = ml_dtypes.bfloat16
    WaHH, Wlo, ob, E, ident, plan, crs = _build_fast_consts(W, b, idx, mask, deltas)
    nc = _build_fast_program(BLOC, crs, plan)

    consts = {"WaHH": WaHH, "WaL": Wlo, "ident": ident}
    for c in range(len(crs)):
        consts[f"ob{c}"] = ob[c]
        consts[f"E{c}"] = E[c]

    # within each 512-batch block, batch column position k' = j*128 + p holds
    # batch row bs + 4p + j  (so the transposed output tile maps partition p,
    # section j to DRAM row blk*512 + 4p + j = contiguous [128, 4*6561] DMA)
    k = np.arange(BF)
    perm_blk = 4 * (k % P) + (k // P)
    perm = (np.arange(BLOC // BF)[:, None] * BF + perm_blk[None, :]).reshape(-1)

    in_maps = []
    for i in range(NCORES):
        shard = obs[i * BLOC:(i + 1) * BLOC, :D]
        xTa = np.concatenate(
            [np.ascontiguousarray(shard.T), np.ones((1, BLOC), np.float32)], axis=0
        )
        xTa = xTa[:, perm]
        xhi = xTa.astype(bf)
        xlo = (xTa - xhi.astype(np.float32)).astype(bf)
        m = dict(consts)
        m["xcat"] = np.ascontiguousarray(np.concatenate([xhi, xlo], axis=0))
        in_maps.append(m)

    br = run_bass_kernel_spmd(nc, in_maps, list(range(NCORES)), trace=trace)
    LAST_RESULTS = br
    out = np.concatenate(
        [
            np.asarray(br.results[i]["out"]).reshape(BLOC, NN).astype(np.float32)
            for i in range(NCORES)
        ],
        axis=0,
    )
    return out.reshape(BATCH, NZ, NZ)


# --------------------------------------------------------------------------
# FALLBACK: original fp32 scatter-matmul kernel (handles arbitrary idx/mask,
# including duplicate targets that must accumulate)
# --------------------------------------------------------------------------

ZPG = 6                        # zones per scatter group (30 slots + 2 pad)
NGRP = 14                      # groups: 13x6 zones + 1x3 zones
GRP_NZ = [6] * 13 + [3]
GRP_COL = [486 * g for g in range(14)]          # output column offset
PW_PAIR = [128, 128, 128, 64]  # used rows per pair (pair 3 = one half-chunk)
PADW = 448                     # 3*128 + 64 packed columns


def _slot(n, k):
    g = n // ZPG
    zz = n % ZPG
    hc = g // 2
    p = hc // 2
    row_hi = 32 * (g % 2) + KADJ * zz + k
    row_pair = 64 * (hc % 2) + row_hi
    return p, row_pair, hc, row_hi


def _build_consts(W, b, idx, mask):
    import ml_dtypes

    bf = ml_dtypes.bfloat16
    W = np.asarray(W, np.float32)
    b = np.asarray(b, np.float32)
    idx = np.asarray(idx)
    mask = np.asarray(mask, np.float32)

    Wa = np.zeros((DA, PADW), np.float32)
    E = np.zeros((NZ, PADW), bf)
    ob = [np.zeros((PW_PAIR[p], NZ), np.float32) for p in range(4)]
    S = np.zeros((P, NZ * NZ), bf)

    for n in range(NZ):
        for k in range(KADJ):
            p, rp, hc, rh = _slot(n, k)
            col = 128 * p + rp
            if mask[n, k] > 0:
                Wa[:D, col] = W[n, :, k]
                Wa[D, col] = b[n, k]
            else:
                Wa[D, col] = NEG
            E[n, col] = 1.0
            ob[p][rp, n] = 1.0
            ocol = n * NZ + int(idx[n, k])
            S[rh, ocol] = 1.0        # hi rows
            S[64 + rh, ocol] = 1.0   # lo rows
    return Wa, E, ob, S


def _build_program(bloc):
    from concourse import bacc, mybir
    import concourse.tile as tile

    f32 = mybir.dt.float32
    bf16 = mybir.dt.bfloat16
    AF = mybir.ActivationFunctionType
    OP = mybir.AluOpType
    nc = bacc.Bacc("TRN2", target_bir_lowering=False, debug=False)

    xTa_d = nc.declare_dram_parameter("xTa", [DA, bloc], f32, isOutput=False)
    Wa_d = nc.declare_dram_parameter("Wa", [DA, PADW], f32, isOutput=False)
    E_d = nc.declare_dram_parameter("E", [NZ, PADW], bf16, isOutput=False)
    ob_d = [
        nc.declare_dram_parameter(f"ob{p}", [PW_PAIR[p], NZ], f32, isOutput=False)
        for p in range(4)
    ]
    S_d = nc.declare_dram_parameter("S", [P, NZ * NZ], bf16, isOutput=False)
    out_d = nc.declare_dram_parameter("out", [bloc, NZ * NZ], f32, isOutput=True)

    n_blk = bloc // BF
    n_sub = BF // P

    with tile.TileContext(nc) as tc:
        with (
            tc.tile_pool(name="const", bufs=1) as cpool,
            tc.tile_pool(name="work", bufs=2) as wpool,
            tc.tile_pool(name="outp", bufs=4) as opool,
            tc.tile_pool(name="ps_log", bufs=2, space="PSUM") as ps_log,
            tc.tile_pool(name="ps_den", bufs=1, space="PSUM") as ps_den,
            tc.tile_pool(name="ps_rf", bufs=2, space="PSUM") as ps_rf,
            tc.tile_pool(name="ps_sc", bufs=3, space="PSUM") as ps_sc,
        ):
            Wa_sb = cpool.tile([DA, PADW], f32, tag="Wa")
            nc.sync.dma_start(out=Wa_sb[:], in_=Wa_d[:])
            E_sb = cpool.tile([NZ, PADW], bf16, tag="E")
            nc.sync.dma_start(out=E_sb[:], in_=E_d[:])
            S_sb = cpool.tile([P, NZ * NZ], bf16, tag="S")
            nc.sync.dma_start(out=S_sb[:], in_=S_d[:])
            ob_sb = []
            for p in range(4):
                t = cpool.tile([PW_PAIR[p], NZ], f32, tag=f"ob{p}")
                nc.sync.dma_start(out=t[:], in_=ob_d[p][:])
                ob_sb.append(t)
            xTa_sb = cpool.tile([DA, bloc], f32, tag="xTa")
            nc.sync.dma_start(out=xTa_sb[:], in_=xTa_d[:])

            def emit_scatter(bs, pcat):
                for i in range(n_sub):
                    osb = opool.tile([P, NZ * NZ], f32, tag="osb")
                    for g in range(NGRP):
                        ncols = GRP_NZ[g] * NZ
                        colg = GRP_COL[g]
                        sc = ps_sc.tile([P, BF], f32, tag="scps")
                        nc.tensor.matmul(
                            sc[:, :ncols],
                            pcat[g // 2][:, i * P:(i + 1) * P],
                            S_sb[:, colg:colg + ncols],
                            start=True,
                            stop=True,
                        )
                        dst = osb[:, colg:colg + ncols]
                        if g % 5 < 3:
                            nc.scalar.copy(dst, sc[:, :ncols])
                        else:
                            nc.vector.tensor_copy(dst, sc[:, :ncols])
                    nc.sync.dma_start(
                        out=out_d[bs + i * P: bs + (i + 1) * P, :], in_=osb[:]
                    )

            prev = None
            for blk in range(n_blk):
                bs = blk * BF
                exT = []
                for p in range(4):
                    pw = PW_PAIR[p]
                    lg = ps_log.tile([P, BF], f32, tag="lg")
                    nc.tensor.matmul(
                        lg[:pw, :],
                        Wa_sb[:, 128 * p:128 * p + pw],
                        xTa_sb[:, bs:bs + BF],
                        start=True,
                        stop=True,
                    )
                    ex = wpool.tile([P, BF], f32, tag=f"exp{p}")
                    nc.scalar.activation(ex[:pw, :], lg[:pw, :], AF.Exp)
                    exT.append(ex)
                den_ps = ps_den.tile([NZ, BF], f32, tag="den")
                for p in range(4):
                    nc.tensor.matmul(
                        den_ps[:, :], ob_sb[p][:], exT[p][:PW_PAIR[p], :],
                        start=(p == 0), stop=(p == 3),
                    )
                rc = wpool.tile([NZ, BF], f32, tag="recipC")
                nc.vector.reciprocal(rc[:], den_ps[:])
                rhi = wpool.tile([NZ, BF], bf16, tag="rhi")
                nc.scalar.copy(rhi[:], rc[:])
                rlo = wpool.tile([NZ, BF], bf16, tag="rlo")
                nc.vector.tensor_tensor(out=rlo[:], in0=rc[:], in1=rhi[:], op=OP.subtract)
                pcat = []
                for p in range(4):
                    pw = PW_PAIR[p]
                    rf = ps_rf.tile([P, BF], f32, tag="rf")
                    nc.tensor.matmul(
                        rf[:pw, :], E_sb[:, 128 * p:128 * p + pw], rhi[:],
                        start=True, stop=False,
                    )
                    nc.tensor.matmul(
                        rf[:pw, :], E_sb[:, 128 * p:128 * p + pw], rlo[:],
                        start=False, stop=True,
                    )
                    for h in range(2 if pw == 128 else 1):
                        sl = slice(64 * h, 64 * h + 64)
                        pt = wpool.tile([64, BF], f32, tag=f"pt{2 * p + h}")
                        nc.vector.tensor_tensor(
                            out=pt[:, :], in0=exT[p][sl, :], in1=rf[sl, :], op=OP.mult
                        )
                        pc = wpool.tile([P, BF], bf16, tag=f"pcat{2 * p + h}")
                        nc.scalar.copy(pc[:64, :], pt[:, :])
                        nc.vector.tensor_tensor(
                            out=pc[64:, :],
                            in0=pt[:, :],
                            in1=pc[:64, :],
                            op=OP.subtract,
                        )
                        pcat.append(pc)
                if prev is not None:
                    emit_scatter(*prev)
                prev = (bs, pcat)
            emit_scatter(*prev)
    nc.compile()
    return nc


def _run_fallback(obs, W, b, idx, mask, trace):
    from concourse.bass_utils import run_bass_kernel_spmd

    global LAST_RESULTS
    Wa, E, ob, S = _build_consts(W, b, idx, mask)
    nc = _build_program(BLOC)

    consts = {"Wa": Wa, "E": E, "S": S}
    for p in range(4):
        consts[f"ob{p}"] = ob[p]

    in_maps = []
    for i in range(NCORES):
        shard = obs[i * BLOC:(i + 1) * BLOC, :D]
        xTa = np.concatenate(
            [np.ascontiguousarray(shard.T), np.ones((1, BLOC), np.float32)], axis=0
        )
        m = dict(consts)
        m["xTa"] = np.ascontiguousarray(xTa)
        in_maps.append(m)

    br = run_bass_kernel_spmd(nc, in_maps, list(range(NCORES)), trace=trace)
    LAST_RESULTS = br
    out = np.concatenate([br.results[i]["out"] for i in range(NCORES)], axis=0)
    return out.reshape(BATCH, NZ, NZ)


# --------------------------------------------------------------------------


def _install_ntff_hook():
    """Shim antenv.axon_hooks (absent in this image) so trace=True can drive
    NRT profiling through libaxon_pjrt.so. Only used for self-profiling."""
    import types

    try:
        import antenv

        try:
            from antenv.axon_hooks import get_axon_ntff_profile_hook  # noqa: F401

            return True
        except ImportError:
            pass
        if "/root/.axon_site" not in sys.path:
            sys.path.insert(0, "/root/.axon_site")
        from trn_agent_boot.trn_boot import _ntff_profile_via_ctypes

        hook = _ntff_profile_via_ctypes("/opt/axon/libaxon_pjrt.so")
        mod = types.ModuleType("antenv.axon_hooks")
        state = {"hook": hook}
        mod.get_axon_ntff_profile_hook = lambda: state["hook"]
        mod.set_axon_ntff_profile_hook = lambda h: state.update(hook=h)
        sys.modules["antenv.axon_hooks"] = mod
        antenv.axon_hooks = mod
        return hook is not None
    except Exception as e:  # profiling is best-effort; never break the run
        print("ntff hook install failed:", e)
        return False


def kernel(obs, W, b, idx, mask):
    trace = bool(int(os.environ.get("KBT_TRACE", "0")))
    if trace:
        trace = _install_ntff_hook()
    obs = np.asarray(obs, np.float32)
    deltas = _analyze_classes(idx, mask)
    if deltas is not None and not bool(int(os.environ.get("KBT_FORCE_FALLBACK", "0"))):
        return _run_fast(obs, W, b, idx, mask, deltas, trace)
    return _run_fallback(obs, W, b, idx, mask, trace)


# revision 7
# speedup vs baseline: 1.8293x; 1.1261x over previous
"""Trainium2 Bass kernel for nn_CollectiveDecActorTaxi0Obs (gnn_message_passing).

Computes, for obs [32768, 48], per-zone dense heads W [81, 48, 5] (+bias b,
adjacency idx/mask [81, 5]):
    logits = einsum('bd,ndk->bnk', obs, W) + b ; masked softmax over k
    out[b, n, idx[n, k]] += probs[b, n, k]              -> [32768, 81, 81] f32

Strategy (pure data parallelism, 8 cores, batch-sharded 4096 rows each):

FAST PATH (taken when idx has diagonal structure, i.e. the per-zone offsets
delta = idx[n,k] - n fall into a small shared class set with no duplicate
targets -- true for the 9x9 grid adjacency: delta in {-9,-1,0,+1,+9}):
  * Slots are laid out (class, zone): slot s = c*81 + n. Absent/masked slots
    get bias -1e9 so exp underflows to exactly 0 (matching the reference's
    where(mask>0, ., -1e9) + softmax*mask).
  * logits via two bf16 matmuls (hi/lo split of both x and W) accumulating in
    PSUM -- full fp32-grade accuracy at 1/2 the fp32 matmul cost.
  * softmax denominator: 0/1 "zone-of-slot" matmul (fp32); reciprocal on DVE;
    expanded back to slots by a 0/1 bf16 matmul.
  * probs = exp * recip (DVE), rounded to bf16.
  * PE transpose (vs identity) flips each [slots,128-batch] tile to
    [128-batch, slots] in PSUM.
  * The scatter out[.., n*81+idx] = probs is 5 strided copies per 128-batch
    sub-block: destination column 81*n + idx[n,k] = 82*n + delta, i.e. one
    stride-82 "diagonal" per class. Zero columns of the output tile are
    memset ONCE and never touched again (same columns rewritten every
    iteration), killing the full-width PSUM evacuation the f32 scatter-matmul
    approach needs.
  * Output is stored to DRAM in bf16 (rel err ~2^-9, far under the 2e-2
    tolerance) and upcast to f32 on the host: halves the HBM write traffic,
    which is the roofline for this problem (430 MB @ ~358 GB/s/core).
  * Output DMAs are [128, 4*6561] = 6.7 MB contiguous per 512-batch block
    (batch row = blk*512 + 4*p + j; the batch columns of the on-chip
    transposed layout are pre-permuted on the host so this layout falls out).

FALLBACK (any other idx/mask structure, incl. duplicate targets that must
accumulate): the original fp32 scatter-matmul kernel (probs @ 0/1 S matrix).
"""

import os
import sys

sys.path.insert(0, "/opt/trn_rl_repo")

import numpy as np

NZ = 81          # zones
D = 48           # obs dim used
DA = D + 1       # + bias row
KADJ = 5         # adjacency slots per zone
NCORES = 8
BATCH = 32768
BLOC = BATCH // NCORES   # 4096 rows per core
BF = 512                 # batch free-dim block
P = 128
NEG = np.float32(-1e9)
NN = NZ * NZ             # 6561

LAST_RESULTS = None


# --------------------------------------------------------------------------
# FAST PATH: diagonal-class scatter
# --------------------------------------------------------------------------

def _analyze_classes(idx, mask):
    """Return sorted class offsets (delta = idx - zone) if the adjacency has
    exploitable diagonal structure, else None."""
    idx = np.asarray(idx)
    mask = np.asarray(mask, np.float32)
    deltas = set()
    for n in range(NZ):
        tgts = set()
        nvalid = 0
        for k in range(KADJ):
            if mask[n, k] > 0:
                t = int(idx[n, k])
                if t in tgts:
                    return None          # duplicate target -> must accumulate
                tgts.add(t)
                deltas.add(t - n)
                nvalid += 1
        if nvalid == 0:
            return None                  # zone with no valid slot (den == 0)
    ds = sorted(deltas)
    if len(ds) > 6 or len(ds) * NZ > 512:
        return None
    for a in ds:
        for b in ds:
            if abs(a - b) == 82:         # column-ownership collision
                return None
    return ds


def _build_fast_consts(W, b, idx, mask, deltas):
    import ml_dtypes

    bf = ml_dtypes.bfloat16
    W = np.asarray(W, np.float32)
    b = np.asarray(b, np.float32)
    idx = np.asarray(idx)
    mask = np.asarray(mask, np.float32)

    ncls = len(deltas)
    nslot = ncls * NZ
    nchunk = (nslot + P - 1) // P
    crs = [min(P, nslot - c * P) for c in range(nchunk)]

    # Wa [DA, nslot] fp32: per-slot weight column + bias row
    Wa = np.zeros((DA, nslot), np.float32)
    Wa[D, :] = NEG                        # default: absent slot -> -1e9 bias
    d2c = {d: c for c, d in enumerate(deltas)}
    for n in range(NZ):
        for k in range(KADJ):
            if mask[n, k] > 0:
                s = d2c[int(idx[n, k]) - n] * NZ + n
                Wa[:D, s] = W[n, :, k]
                Wa[D, s] = b[n, k]

    Whi = Wa.astype(bf)
    Wlo = (Wa - Whi.astype(np.float32)).astype(bf)
    WaHH = np.concatenate([Whi, Whi], axis=0)          # [2*DA, nslot] bf16

    # ob chunks [cr, NZ] f32 (zone-of-slot, for the fp32 denominator matmul)
    # E  chunks [NZ, cr] bf16 (slot-of-zone, to expand recip back to slots)
    ob = []
    E = []
    for c in range(nchunk):
        cr = crs[c]
        o = np.zeros((cr, NZ), np.float32)
        e = np.zeros((NZ, cr), bf)
        for l in range(cr):
            n = (c * P + l) % NZ
            o[l, n] = 1.0
            e[n, l] = 1.0
        ob.append(o)
        E.append(e)

    ident = np.eye(P, dtype=np.float32).astype(bf)

    # scatter plan: per class, the n-range whose dest column 82n+delta is
    # in [0, NN)
    plan = []
    for c, dlt in enumerate(deltas):
        n0 = 1 if dlt < 0 else 0
        n1 = 80 if dlt > 0 else 81
        plan.append((c, dlt, n0, n1))
    return WaHH, Wlo, ob, E, ident, plan, crs


def _build_fast_program(bloc, crs, plan):
    from concourse import bacc, mybir
    import concourse.tile as tile

    f32 = mybir.dt.float32
    bf16 = mybir.dt.bfloat16
    AF = mybir.ActivationFunctionType
    OP = mybir.AluOpType
    nc = bacc.Bacc("TRN2", target_bir_lowering=False, debug=False)

    nchunk = len(crs)
    nslot = sum(crs)
    DH = 2 * DA                                   # 98 rows: x hi + x lo
    n_blk = bloc // BF

    xcat_d = nc.declare_dram_parameter("xcat", [DH, bloc], bf16, isOutput=False)
    WaHH_d = nc.declare_dram_parameter("WaHH", [DH, nslot], bf16, isOutput=False)
    WaL_d = nc.declare_dram_parameter("WaL", [DA, nslot], bf16, isOutput=False)
    ob_d = [
        nc.declare_dram_parameter(f"ob{c}", [crs[c], NZ], f32, isOutput=False)
        for c in range(nchunk)
    ]
    E_d = [
        nc.declare_dram_parameter(f"E{c}", [NZ, crs[c]], bf16, isOutput=False)
        for c in range(nchunk)
    ]
    id_d = nc.declare_dram_parameter("ident", [P, P], bf16, isOutput=False)
    out_d = nc.declare_dram_parameter("out", [n_blk, P, 4 * NN], bf16, isOutput=True)

    with tile.TileContext(nc) as tc:
        with (
            tc.tile_pool(name="const", bufs=1) as cpool,
            tc.tile_pool(name="work", bufs=2) as wpool,
            tc.tile_pool(name="ps_log", bufs=2, space="PSUM") as ps_log,
            tc.tile_pool(name="ps_den", bufs=2, space="PSUM") as ps_den,
            tc.tile_pool(name="ps_rf", bufs=2, space="PSUM") as ps_rf,
            tc.tile_pool(name="ps_t", bufs=2, space="PSUM") as ps_t,
        ):
            WaHH_sb = cpool.tile([DH, nslot], bf16, tag="WaHH")
            nc.sync.dma_start(out=WaHH_sb[:], in_=WaHH_d[:])
            WaL_sb = cpool.tile([DA, nslot], bf16, tag="WaL")
            nc.sync.dma_start(out=WaL_sb[:], in_=WaL_d[:])
            # xcat loaded per 512-batch block so block 0 can start ~immediately
            xcat_sb = cpool.tile([DH, bloc], bf16, tag="xcat")
            for blk in range(n_blk):
                bs = blk * BF
                nc.sync.dma_start(
                    out=xcat_sb[:, bs:bs + BF], in_=xcat_d[:, bs:bs + BF]
                )
            ob_sb, E_sb = [], []
            for c in range(nchunk):
                t = cpool.tile([crs[c], NZ], f32, tag=f"ob{c}")
                nc.sync.dma_start(out=t[:], in_=ob_d[c][:])
                ob_sb.append(t)
                t = cpool.tile([NZ, crs[c]], bf16, tag=f"E{c}")
                nc.sync.dma_start(out=t[:], in_=E_d[c][:])
                E_sb.append(t)
            id_sb = cpool.tile([P, P], bf16, tag="ident")
            nc.sync.dma_start(out=id_sb[:], in_=id_d[:])

            # zero-fill the two output staging buffers: their non-scatter
            # columns are written once here and never touched again.
            # Split across engines so neither buffer gates the pipeline.
            ZC = 1458  # 26244 / 18
            zrow = cpool.tile([P, ZC], bf16, tag="zrow")
            nc.vector.memset(zrow[:], 0.0)
            osbA = cpool.tile([P, 4 * NN], bf16, tag="osbA")
            for z in range(18):
                nc.vector.tensor_copy(osbA[:, z * ZC:(z + 1) * ZC], zrow[:])
            osbB = cpool.tile([P, 4 * NN], bf16, tag="osbB")
            nc.gpsimd.memset(osbB[:, :2 * NN], 0.0)
            for z in range(9, 18):
                nc.scalar.copy(osbB[:, z * ZC:(z + 1) * ZC], zrow[:])
            osbs = [osbA, osbB]

            for blk in range(n_blk):
                bs = blk * BF
                osb = osbs[blk % 2]
                exs = []
                for c in range(nchunk):
                    cr = crs[c]
                    sl = slice(c * P, c * P + cr)
                    lg = ps_log.tile([P, BF], f32, tag="lg")
                    nc.tensor.matmul(
                        lg[:cr, :], WaHH_sb[:, sl], xcat_sb[:, bs:bs + BF],
                        start=True, stop=False,
                    )
                    nc.tensor.matmul(
                        lg[:cr, :], WaL_sb[:, sl], xcat_sb[:DA, bs:bs + BF],
                        start=False, stop=True,
                    )
                    ex = wpool.tile([P, BF], f32, tag=f"ex{c}")
                    nc.scalar.activation(ex[:cr, :], lg[:cr, :], AF.Exp)
                    exs.append(ex)
                den = ps_den.tile([NZ, BF], f32, tag="den")
                for c in range(nchunk):
                    nc.tensor.matmul(
                        den[:, :], ob_sb[c][:], exs[c][:crs[c], :],
                        start=(c == 0), stop=(c == nchunk - 1),
                    )
                rc = wpool.tile([NZ, BF], f32, tag="rc")
                nc.vector.reciprocal_approx_fast(rc[:], den[:])
                rhi = wpool.tile([NZ, BF], bf16, tag="rhi")
                nc.scalar.copy(rhi[:], rc[:])
                pcs = []
                for c in range(nchunk):
                    cr = crs[c]
                    rf = ps_rf.tile([P, BF], f32, tag="rf")
                    nc.tensor.matmul(
                        rf[:cr, :], E_sb[c][:], rhi[:], start=True, stop=True,
                    )
                    pc = wpool.tile([P, BF], bf16, tag=f"pc{c}")
                    nc.vector.tensor_tensor(
                        out=pc[:cr, :], in0=exs[c][:cr, :], in1=rf[:cr, :],
                        op=OP.mult,
                    )
                    pcs.append(pc)
                for j in range(4):
                    pT = ps_t.tile([P, BF], bf16, tag="pT")
                    for c in range(nchunk):
                        cr = crs[c]
                        nc.tensor.transpose(
                            pT[:, c * P:c * P + cr],
                            pcs[c][:cr, j * P:(j + 1) * P],
                            id_sb[:cr, :cr],
                        )
                    for ci, (c, dlt, n0, n1) in enumerate(plan):
                        cnt = n1 - n0
                        src = pT[:, c * NZ + n0:c * NZ + n1]
                        d0 = j * NN + 82 * n0 + dlt
                        dst = osb[:, d0:d0 + 82 * (cnt - 1) + 1:82]
                        if (j + ci) % 2 == 0:
                            nc.vector.tensor_copy(dst, src)
                        else:
                            nc.scalar.copy(dst, src)
                    if blk == 0:
                        # tiny first DMAs: start writing as early as possible
                        eng = nc.sync if j % 2 == 0 else nc.scalar
                        eng.dma_start(
                            out=out_d[blk][:, j * NN:(j + 1) * NN],
                            in_=osb[:, j * NN:(j + 1) * NN],
                        )
                if blk > 0:
                    # two half-block DMAs on the two HWDGE rings
                    nc.sync.dma_start(
                        out=out_d[blk][:, :2 * NN], in_=osb[:, :2 * NN]
                    )
                    nc.scalar.dma_start(
                        out=out_d[blk][:, 2 * NN:], in_=osb[:, 2 * NN:]
                    )
    nc.compile()
    return nc


def _run_fast(obs, W, b, idx, mask, deltas, trace):
    import ml_dtypes
    from concourse.bass_utils import run_bass_kernel_spmd

    global LAST_RESULTS
    bf = ml_dtypes.bfloat16
    WaHH, Wlo, ob, E, ident, plan, crs = _build_fast_consts(W, b, idx, mask, deltas)
    nc = _build_fast_program(BLOC, crs, plan)

    consts = {"WaHH": WaHH, "WaL": Wlo, "ident": ident}
    for c in range(len(crs)):
        consts[f"ob{c}"] = ob[c]
        consts[f"E{c}"] = E[c]

    # within each 512-batch block, batch column position k' = j*128 + p holds
    # batch row bs + 4p + j  (so the transposed output tile maps partition p,
    # section j to DRAM row blk*512 + 4p + j = contiguous [128, 4*6561] DMA)
    k = np.arange(BF)
    perm_blk = 4 * (k % P) + (k // P)
    perm = (np.arange(BLOC // BF)[:, None] * BF + perm_blk[None, :]).reshape(-1)

    in_maps = []
    for i in range(NCORES):
        shard = obs[i * BLOC:(i + 1) * BLOC, :D]
        xTa = np.concatenate(
            [np.ascontiguousarray(shard.T), np.ones((1, BLOC), np.float32)], axis=0
        )
        xTa = xTa[:, perm]
        xhi = xTa.astype(bf)
        xlo = (xTa - xhi.astype(np.float32)).astype(bf)
        m = dict(consts)
        m["xcat"] = np.ascontiguousarray(np.concatenate([xhi, xlo], axis=0))
        in_maps.append(m)

    br = run_bass_kernel_spmd(nc, in_maps, list(range(NCORES)), trace=trace)
    LAST_RESULTS = br
    out = np.concatenate(
        [
            np.asarray(br.results[i]["out"]).reshape(BLOC, NN).astype(np.float32)
            for i in range(NCORES)
        ],
        axis=0,
    )
    return out.reshape(BATCH, NZ, NZ)


# --------------------------------------------------------------------------
# FALLBACK: original fp32 scatter-matmul kernel (handles arbitrary idx/mask,
# including duplicate targets that must accumulate)
# --------------------------------------------------------------------------

ZPG = 6                        # zones per scatter group (30 slots + 2 pad)
NGRP = 14                      # groups: 13x6 zones + 1x3 zones
GRP_NZ = [6] * 13 + [3]
GRP_COL = [486 * g for g in range(14)]          # output column offset
PW_PAIR = [128, 128, 128, 64]  # used rows per pair (pair 3 = one half-chunk)
PADW = 448                     # 3*128 + 64 packed columns


def _slot(n, k):
    g = n // ZPG
    zz = n % ZPG
    hc = g // 2
    p = hc // 2
    row_hi = 32 * (g % 2) + KADJ * zz + k
    row_pair = 64 * (hc % 2) + row_hi
    return p, row_pair, hc, row_hi


def _build_consts(W, b, idx, mask):
    import ml_dtypes

    bf = ml_dtypes.bfloat16
    W = np.asarray(W, np.float32)
    b = np.asarray(b, np.float32)
    idx = np.asarray(idx)
    mask = np.asarray(mask, np.float32)

    Wa = np.zeros((DA, PADW), np.float32)
    E = np.zeros((NZ, PADW), bf)
    ob = [np.zeros((PW_PAIR[p], NZ), np.float32) for p in range(4)]
    S = np.zeros((P, NZ * NZ), bf)

    for n in range(NZ):
        for k in range(KADJ):
            p, rp, hc, rh = _slot(n, k)
            col = 128 * p + rp
            if mask[n, k] > 0:
                Wa[:D, col] = W[n, :, k]
                Wa[D, col] = b[n, k]
            else:
                Wa[D, col] = NEG
            E[n, col] = 1.0
            ob[p][rp, n] = 1.0
            ocol = n * NZ + int(idx[n, k])
            S[rh, ocol] = 1.0        # hi rows
            S[64 + rh, ocol] = 1.0   # lo rows
    return Wa, E, ob, S


def _build_program(bloc):
    from concourse import bacc, mybir
    import concourse.tile as tile

    f32 = mybir.dt.float32
    bf16 = mybir.dt.bfloat16
    AF = mybir.ActivationFunctionType
    OP = mybir.AluOpType
    nc = bacc.Bacc("TRN2", target_bir_lowering=False, debug=False)

    xTa_d = nc.declare_dram_parameter("xTa", [DA, bloc], f32, isOutput=False)
    Wa_d = nc.declare_dram_parameter("Wa", [DA, PADW], f32, isOutput=False)
    E_d = nc.declare_dram_parameter("E", [NZ, PADW], bf16, isOutput=False)
    ob_d = [
        nc.declare_dram_parameter(f"ob{p}", [PW_PAIR[p], NZ], f32, isOutput=False)
        for p in range(4)
    ]
    S_d = nc.declare_dram_parameter("S", [P, NZ * NZ], bf16, isOutput=False)
    out_d = nc.declare_dram_parameter("out", [bloc, NZ * NZ], f32, isOutput=True)

    n_blk = bloc // BF
    n_sub = BF // P

    with tile.TileContext(nc) as tc:
        with (
            tc.tile_pool(name="const", bufs=1) as cpool,
            tc.tile_pool(name="work", bufs=2) as wpool,
            tc.tile_pool(name="outp", bufs=4) as opool,
            tc.tile_pool(name="ps_log", bufs=2, space="PSUM") as ps_log,
            tc.tile_pool(name="ps_den", bufs=1, space="PSUM") as ps_den,
            tc.tile_pool(name="ps_rf", bufs=2, space="PSUM") as ps_rf,
            tc.tile_pool(name="ps_sc", bufs=3, space="PSUM") as ps_sc,
        ):
            Wa_sb = cpool.tile([DA, PADW], f32, tag="Wa")
            nc.sync.dma_start(out=Wa_sb[:], in_=Wa_d[:])
            E_sb = cpool.tile([NZ, PADW], bf16, tag="E")
            nc.sync.dma_start(out=E_sb[:], in_=E_d[:])
            S_sb = cpool.tile([P, NZ * NZ], bf16, tag="S")
            nc.sync.dma_start(out=S_sb[:], in_=S_d[:])
            ob_sb = []
            for p in range(4):
                t = cpool.tile([PW_PAIR[p], NZ], f32, tag=f"ob{p}")
                nc.sync.dma_start(out=t[:], in_=ob_d[p][:])
                ob_sb.append(t)
            xTa_sb = cpool.tile([DA, bloc], f32, tag="xTa")
            nc.sync.dma_start(out=xTa_sb[:], in_=xTa_d[:])

            def emit_scatter(bs, pcat):
                for i in range(n_sub):
                    osb = opool.tile([P, NZ * NZ], f32, tag="osb")
                    for g in range(NGRP):
                        ncols = GRP_NZ[g] * NZ
                        colg = GRP_COL[g]
                        sc = ps_sc.tile([P, BF], f32, tag="scps")
                        nc.tensor.matmul(
                            sc[:, :ncols],
                            pcat[g // 2][:, i * P:(i + 1) * P],
                            S_sb[:, colg:colg + ncols],
                            start=True,
                            stop=True,
                        )
                        dst = osb[:, colg:colg + ncols]
                        if g % 5 < 3:
                            nc.scalar.copy(dst, sc[:, :ncols])
                        else:
                            nc.vector.tensor_copy(dst, sc[:, :ncols])
                    nc.sync.dma_start(
                        out=out_d[bs + i * P: bs + (i + 1) * P, :], in_=osb[:]
                    )

            prev = None
            for blk in range(n_blk):
                bs = blk * BF
                exT = []
                for p in range(4):
                    pw = PW_PAIR[p]
                    lg = ps_log.tile([P, BF], f32, tag="lg")
                    nc.tensor.matmul(
                        lg[:pw, :],
                        Wa_sb[:, 128 * p:128 * p + pw],
                        xTa_sb[:, bs:bs + BF],
                        start=True,
                        stop=True,
                    )
                    ex = wpool.tile([P, BF], f32, tag=f"exp{p}")
                    nc.scalar.activation(ex[:pw, :], lg[:pw, :], AF.Exp)
                    exT.append(ex)
                den_ps = ps_den.tile([NZ, BF], f32, tag="den")
                for p in range(4):
                    nc.tensor.matmul(
                        den_ps[:, :], ob_sb[p][:], exT[p][:PW_PAIR[p], :],
                        start=(p == 0), stop=(p == 3),
                    )
                rc = wpool.tile([NZ, BF], f32, tag="recipC")
                nc.vector.reciprocal(rc[:], den_ps[:])
                rhi = wpool.tile([NZ, BF], bf16, tag="rhi")
                nc.scalar.copy(rhi[:], rc[:])
                rlo = wpool.tile([NZ, BF], bf16, tag="rlo")
                nc.vector.tensor_tensor(out=rlo[:], in0=rc[:], in1=rhi[:], op=OP.subtract)
                pcat = []
                for p in range(4):
                    pw = PW_PAIR[p]
                    rf = ps_rf.tile([P, BF], f32, tag="rf")
                    nc.tensor.matmul(
                        rf[:pw, :], E_sb[:, 128 * p:128 * p + pw], rhi[:],
                        start=True, stop=False,
                    )
                    nc.tensor.matmul(
                        rf[:pw, :], E_sb[:, 128 * p:128 * p + pw], rlo[:],
                        start=False, stop=True,
                    )
                    for h in range(2 if pw == 128 else 1):
                        sl = slice(64 * h, 64 * h + 64)
                        pt = wpool.tile([64, BF], f32, tag=f"pt{2 * p + h}")
                        nc.vector.tensor_tensor(
                            out=pt[:, :], in0=exT[p][sl, :], in1=rf[sl, :], op=OP.mult
                        )
                        pc = wpool.tile([P, BF], bf16, tag=f"pcat{2 * p + h}")
                        nc.scalar.copy(pc[:64, :], pt[:, :])
                        nc.vector.tensor_tensor(
                            out=pc[64:, :],
                            in0=pt[:, :],
                            in1=pc[:64, :],
                            op=OP.subtract,
                        )
                        pcat.append(pc)
                if prev is not None:
                    emit_scatter(*prev)
                prev = (bs, pcat)
            emit_scatter(*prev)
    nc.compile()
    return nc


def _run_fallback(obs, W, b, idx, mask, trace):
    from concourse.bass_utils import run_bass_kernel_spmd

    global LAST_RESULTS
    Wa, E, ob, S = _build_consts(W, b, idx, mask)
    nc = _build_program(BLOC)

    consts = {"Wa": Wa, "E": E, "S": S}
    for p in range(4):
        consts[f"ob{p}"] = ob[p]

    in_maps = []
    for i in range(NCORES):
        shard = obs[i * BLOC:(i + 1) * BLOC, :D]
        xTa = np.concatenate(
            [np.ascontiguousarray(shard.T), np.ones((1, BLOC), np.float32)], axis=0
        )
        m = dict(consts)
        m["xTa"] = np.ascontiguousarray(xTa)
        in_maps.append(m)

    br = run_bass_kernel_spmd(nc, in_maps, list(range(NCORES)), trace=trace)
    LAST_RESULTS = br
    out = np.concatenate([br.results[i]["out"] for i in range(NCORES)], axis=0)
    return out.reshape(BATCH, NZ, NZ)


# --------------------------------------------------------------------------


def _install_ntff_hook():
    """Shim antenv.axon_hooks (absent in this image) so trace=True can drive
    NRT profiling through libaxon_pjrt.so. Only used for self-profiling."""
    import types

    try:
        import antenv

        try:
            from antenv.axon_hooks import get_axon_ntff_profile_hook  # noqa: F401

            return True
        except ImportError:
            pass
        if "/root/.axon_site" not in sys.path:
            sys.path.insert(0, "/root/.axon_site")
        from trn_agent_boot.trn_boot import _ntff_profile_via_ctypes

        hook = _ntff_profile_via_ctypes("/opt/axon/libaxon_pjrt.so")
        mod = types.ModuleType("antenv.axon_hooks")
        state = {"hook": hook}
        mod.get_axon_ntff_profile_hook = lambda: state["hook"]
        mod.set_axon_ntff_profile_hook = lambda h: state.update(hook=h)
        sys.modules["antenv.axon_hooks"] = mod
        antenv.axon_hooks = mod
        return hook is not None
    except Exception as e:  # profiling is best-effort; never break the run
        print("ntff hook install failed:", e)
        return False


def kernel(obs, W, b, idx, mask):
    trace = bool(int(os.environ.get("KBT_TRACE", "0")))
    if trace:
        trace = _install_ntff_hook()
    obs = np.asarray(obs, np.float32)
    deltas = _analyze_classes(idx, mask)
    if deltas is not None and not bool(int(os.environ.get("KBT_FORCE_FALLBACK", "0"))):
        return _run_fast(obs, W, b, idx, mask, deltas, trace)
    return _run_fallback(obs, W, b, idx, mask, trace)
